# revision 2
# baseline (speedup 1.0000x reference)
"""BipartiteSAGEConv Trainium2 kernel (v2).

Strategy: destination-sharded, zero collectives, fully transposed dataflow.
- Host: partition edges by destination across 8 cores (6250 dsts each),
  group per 128-dst tile, split by src half (int16 index limit), pad to
  uniform chunk structure across cores (SPMD: one program, 8 data sets).
  Host also computes 1/deg per dst (index-only preprocessing).
- Device per core: dma_gather pulls per-edge src rows HBM->SBUF (fp16,
  per-(tile,half) gathers so trailing pad indices of -1 are skipped by
  the ucode); scatter-add via one-hot matmul on the TensorEngine with the
  GATHERED ROWS AS STATIONARY so the accumulator is agg^T directly
  (no PE transposes); divide fused into the PSUM eviction (DVE multiply
  by a replicated 1/deg row); two linear layers as W^T @ x matmuls; bias
  fused into the ACT-engine eviction; output stored transposed and
  un-transposed on the host.
"""

import os
import sys
import types

import numpy as np

N_SRC = 50000
N_DST = 50000
E = 800000
D = 128
OUT = 128
N_CORES = 8
P = 128
DST_PER_CORE = N_DST // N_CORES          # 6250
TILES = (DST_PER_CORE + P - 1) // P      # 49
LAST_COLS = DST_PER_CORE - (TILES - 1) * P  # 106
HALF = 25000                             # int16 index limit split
MAX_ROWS = int(os.environ.get("BSAGE_ROWS", "1024"))  # rows per gather inst
GROUP = 2                                # tiles per psum group
SKIP_PAD = os.environ.get("BSAGE_SKIP", "1") == "1"


def _install_ntff_hook():
    try:
        import antenv
        if "antenv.axon_hooks" in sys.modules:
            return
        mod = types.ModuleType("antenv.axon_hooks")
        _h = [None]
        mod.set_axon_ntff_profile_hook = lambda h: _h.__setitem__(0, h)
        mod.get_axon_ntff_profile_hook = lambda: _h[0]
        sys.modules["antenv.axon_hooks"] = mod
        antenv.axon_hooks = mod
        from trn_agent_boot.trn_boot import _ntff_profile_via_ctypes
        mod.set_axon_ntff_profile_hook(
            _ntff_profile_via_ctypes("/opt/axon/libaxon_pjrt.so"))
    except Exception:
        pass


def _prep_core(edge_src, edge_dst, core):
    """Per-core: for each (tile, half) the (src, dstl) edge lists."""
    lo = core * DST_PER_CORE
    m = (edge_dst >= lo) & (edge_dst < lo + DST_PER_CORE)
    es = edge_src[m]
    ed = edge_dst[m] - lo
    order = np.argsort(ed, kind="stable")
    es, ed = es[order], ed[order]
    tiles = []
    tile_id = ed >> 7
    bounds = np.searchsorted(tile_id, np.arange(TILES + 1))
    for t in range(TILES):
        a, b = bounds[t], bounds[t + 1]
        s, dl = es[a:b], ed[a:b] - t * P
        is_lo = s < HALF
        tiles.append((s[is_lo], dl[is_lo], s[~is_lo] - HALF, dl[~is_lo]))
    return tiles


def _wrap_idx(idx_flat):
    """dma_gather wrapped index layout: index j at partition j%16, col j//16,
    replicated across the 8 gpsimd cores (partition groups of 16)."""
    n = len(idx_flat)
    w = idx_flat.reshape(n // 16, 16).T          # [16, n/16]
    return np.tile(w, (8, 1))                    # [128, n/16]


def build_and_run(x_src, x_dst, edge_src, edge_dst, W_neigh, b_neigh,
                  W_self, b_self):
    _install_ntff_hook()
    from concourse import bacc, bass, mybir, tile
    from concourse.bass_utils import run_bass_kernel_spmd

    F32 = mybir.dt.float32
    F16 = mybir.dt.float16

    # ---------- host-side sharding / layout ----------
    per_core_tiles = [_prep_core(edge_src, edge_dst, c) for c in range(N_CORES)]

    # uniform chunk counts across cores (SPMD: one program, 8 data sets)
    KL = [max(max(1, -(-len(per_core_tiles[c][t][0]) // P))
              for c in range(N_CORES)) for t in range(TILES)]
    KH = [max(max(1, -(-len(per_core_tiles[c][t][2]) // P))
              for c in range(N_CORES)) for t in range(TILES)]
    # gather row count per (tile, half): rounded up to 16, uniform over cores
    def _r16(n):
        return max(16, -(-n // 16) * 16)
    RL = [_r16(max(len(per_core_tiles[c][t][0]) for c in range(N_CORES)))
          for t in range(TILES)]
    RH = [_r16(max(len(per_core_tiles[c][t][2]) for c in range(N_CORES)))
          for t in range(TILES)]
    if not SKIP_PAD:
        RL = [KL[t] * P for t in range(TILES)]
        RH = [KH[t] * P for t in range(TILES)]

    # tile groups sharing one g buffer / psum accumulator
    GROUPS = [list(range(g, min(g + GROUP, TILES)))
              for g in range(0, TILES, GROUP)]
    # chunk layout within a group buffer: [tA_lo | tA_hi | tB_lo | tB_hi]
    chunk_off = {}
    group_chunks = []
    for gi, gts in enumerate(GROUPS):
        off = 0
        for t in gts:
            chunk_off[(t, 0)] = off
            off += KL[t]
            chunk_off[(t, 1)] = off
            off += KH[t]
        group_chunks.append(off)
    GCMAX = max(group_chunks)
    NCH = sum(KL) + sum(KH)
    cbase_t = {}
    _c = 0
    for gi, gts in enumerate(GROUPS):
        for t in gts:
            cbase_t[(t, 0)] = _c
            _c += KL[t]
            cbase_t[(t, 1)] = _c
            _c += KH[t]
    gbase = np.concatenate([[0], np.cumsum(group_chunks)])

    # gather plan: per (tile, half), split into <= MAX_ROWS-row instructions.
    # entries: (group, half, tile, chunk_off_in_group, n_chunks, idx_col_base,
    #           n_valid_rows)
    gathers = []
    idx_cols = 0
    for gi, gts in enumerate(GROUPS):
        for t in gts:
            for half in (0, 1):
                k_tot = KL[t] if half == 0 else KH[t]
                r_tot = RL[t] if half == 0 else RH[t]
                base = chunk_off[(t, half)]
                k_done = 0
                while k_done < k_tot:
                    k = min(k_tot - k_done, MAX_ROWS // P)
                    valid = min(max(0, r_tot - k_done * P), k * P)
                    gathers.append((gi, half, t, base + k_done, k, idx_cols,
                                    valid))
                    idx_cols += k * 8
                    k_done += k
    IDXCOLS = idx_cols

    # per-core data arrays
    idx_all = np.zeros((N_CORES, P, IDXCOLS), np.int16)
    dstl_all = np.zeros((N_CORES, P, NCH), np.float32)
    for c in range(N_CORES):
        for t in range(TILES):
            s_lo, d_lo, s_hi, d_hi = per_core_tiles[c][t]
            for half, (s, dl, K, R) in enumerate(
                    [(s_lo, d_lo, KL[t], RL[t]), (s_hi, d_hi, KH[t], RH[t])]):
                n = K * P
                # idx: [actual srcs][0-pad to R][-1 to n]
                s_pad = np.full(n, -1, np.int16)
                s_pad[:R] = 0
                s_pad[:len(s)] = s.astype(np.int16)
                d_pad = np.full(n, -1.0, np.float32)
                d_pad[:len(dl)] = dl.astype(np.float32)
                cb = cbase_t[(t, half)]
                dstl_all[c][:, cb:cb + K] = d_pad.reshape(K, P).T
                # write wrapped idx for the gather instructions of this span
                for (gi, h2, t2, off, k, colb, valid) in gathers:
                    if t2 != t or h2 != half:
                        continue
                    k0 = off - chunk_off[(t, half)]
                    rows = s_pad[k0 * P:(k0 + k) * P]
                    idx_all[c][:, colb:colb + k * 8] = _wrap_idx(rows)

    x_lo = np.ascontiguousarray(x_src[:HALF]).astype(np.float16)
    x_hi = np.ascontiguousarray(x_src[HALF:]).astype(np.float16)
    # x_dst transposed per core shard, fp16
    xdstT = np.zeros((N_CORES, P, TILES * P), np.float16)
    for c in range(N_CORES):
        shard = x_dst[c * DST_PER_CORE:(c + 1) * DST_PER_CORE]
        xdstT[c][:, :DST_PER_CORE] = shard.T.astype(np.float16)
    # 1/deg per dst (host: pure index preprocessing), padded row per core
    cnt = np.bincount(edge_dst.astype(np.int64), minlength=N_DST).astype(
        np.float32)
    rcnt = 1.0 / np.maximum(cnt, 1.0)
    rcnt_row = np.ones((N_CORES, 1, TILES * P), np.float32)
    for c in range(N_CORES):
        rcnt_row[c][0, :DST_PER_CORE] = rcnt[c * DST_PER_CORE:
                                             (c + 1) * DST_PER_CORE]
    iota = np.tile(np.arange(P, dtype=np.float32), (P, 1))
    wn = W_neigh.astype(np.float16)
    ws = W_self.astype(np.float16)
    bsumT = (b_neigh + b_self).astype(np.float32)[:, None]  # [128,1]

    # idx column split: first group's gathers load first (early start)
    cols_g0 = max((g[5] + g[4] * 8) for g in gathers if g[0] == 0)

    # ---------- device program ----------
    nc = bacc.Bacc("TRN2", target_bir_lowering=False, debug=False,
                   num_devices=N_CORES, num_swdge_queues=4)
    xlo_d = nc.dram_tensor("xlo", [HALF, D], F16, kind="ExternalInput").ap()
    xhi_d = nc.dram_tensor("xhi", [HALF, D], F16, kind="ExternalInput").ap()
    idx_d = nc.dram_tensor("idx", [P, IDXCOLS], mybir.dt.int16,
                           kind="ExternalInput").ap()
    dstl_d = nc.dram_tensor("dstl", [P, NCH], F32, kind="ExternalInput").ap()
    xdstT_d = nc.dram_tensor("xdstT", [P, TILES * P], F16,
                             kind="ExternalInput").ap()
    iota_d = nc.dram_tensor("iota", [P, P], F32, kind="ExternalInput").ap()
    rcr_d = nc.dram_tensor("rcr", [1, TILES * P], F32,
                           kind="ExternalInput").ap()
    wn_d = nc.dram_tensor("wn", [D, OUT], F16, kind="ExternalInput").ap()
    ws_d = nc.dram_tensor("ws", [D, OUT], F16, kind="ExternalInput").ap()
    bsumT_d = nc.dram_tensor("bsumT", [OUT, 1], F32, kind="ExternalInput").ap()
    out_d = nc.dram_tensor("out", [P, TILES * P], F32,
                           kind="ExternalOutput").ap()

    RTILES = -(-TILES * P // 512)  # rcnt replicate steps of 512 cols

    with tile.TileContext(nc) as tc:
        with (
            tc.tile_pool(name="const", bufs=1) as cpool,
            tc.tile_pool(name="work", bufs=4) as wpool,
            tc.tile_pool(name="psum", bufs=2, space="PSUM") as ppool,
        ):
            idx_sb = cpool.tile([P, IDXCOLS], mybir.dt.int16)
            dstl_sb = cpool.tile([P, NCH], F32)
            xdstT_sb = cpool.tile([P, TILES * P], F16)
            iota_sb = cpool.tile([P, P], F32)
            rcr_sb = cpool.tile([1, TILES * P], F32)
            rcrep_sb = cpool.tile([P, TILES * P], F32)
            wn_sb = cpool.tile([D, OUT], F16)
            ws_sb = cpool.tile([D, OUT], F16)
            bsumT_sb = cpool.tile([OUT, 1], F32)
            ones_sb = cpool.tile([1, P], F32)

            # critical path first: group-0 idx, then the rest
            nc.sync.dma_start(out=idx_sb[:, :cols_g0], in_=idx_d[:, :cols_g0])
            nc.sync.dma_start(out=idx_sb[:, cols_g0:], in_=idx_d[:, cols_g0:])
            nc.sync.dma_start(out=dstl_sb[:], in_=dstl_d[:])
            nc.sync.dma_start(out=iota_sb[:], in_=iota_d[:])
            nc.scalar.dma_start(out=rcr_sb[:], in_=rcr_d[:])
            nc.scalar.dma_start(out=wn_sb[:], in_=wn_d[:])
            nc.scalar.dma_start(out=ws_sb[:], in_=ws_d[:])
            nc.scalar.dma_start(out=bsumT_sb[:], in_=bsumT_d[:])
            nc.scalar.dma_start(out=xdstT_sb[:], in_=xdstT_d[:])
            nc.vector.memset(ones_sb[:], 1.0)

            # replicate rcnt row across partitions via PE (ones^T @ rcr)
            for j in range(RTILES):
                a = j * 512
                b = min((j + 1) * 512, TILES * P)
                psr = ppool.tile([P, 512], F32, tag="psr", name=f"psr{j}",
                                 space="PSUM")
                nc.tensor.matmul(out=psr[:, :b - a], lhsT=ones_sb[:],
                                 rhs=rcr_sb[:, a:b], start=True, stop=True)
                nc.scalar.copy(out=rcrep_sb[:, a:b], in_=psr[:, :b - a])

            g_by_group = [[] for _ in range(len(GROUPS))]
            for g in gathers:
                g_by_group[g[0]].append(g)

            gq = [0]
            for gi, gts in enumerate(GROUPS):
                gcols = group_chunks[gi] * P
                g_sb = wpool.tile([P, GCMAX * P], F16, tag="g", name=f"g{gi}",
                                  bufs=3)
                # memset once per buffer generation to avoid NaN in skipped
                # trailing rows (one-hot is 0 there; 0*NaN would poison psum)
                if gi < 3:
                    nc.vector.memset(g_sb[:, :gcols], 0.0)
                for (_, half, t, off, k, colb, valid) in g_by_group[gi]:
                    t_ap = g_sb[:]
                    out3d = bass.AP(t_ap.tensor, t_ap.offset + off * P,
                                    [t_ap.ap[0], [P, k], [1, P]])
                    nc.gpsimd.dma_gather(
                        out3d,
                        (xlo_d if half == 0 else xhi_d)[:],
                        idx_sb[:, colb:colb + k * 8],
                        k * P,
                        valid,
                        D,
                        queue_num=(gq[0] % 4),
                    )
                    gq[0] += 1

                # one-hot for the whole group in one DVE instruction
                kq = group_chunks[gi]
                oh_sb = wpool.tile([P, GCMAX * P], F16, tag="oh",
                                   name=f"oh{gi}", bufs=3)
                i_ap = iota_sb[:]
                iota3d = bass.AP(i_ap.tensor, i_ap.offset,
                                 [i_ap.ap[0], [0, kq], [i_ap.ap[1][0], P]])
                d_ap = dstl_sb[:]
                dstl3d = bass.AP(d_ap.tensor, d_ap.offset + int(gbase[gi]),
                                 [d_ap.ap[0], [d_ap.ap[1][0], kq], [0, P]])
                oh3d = bass.AP(oh_sb[:].tensor, oh_sb[:].offset,
                               [oh_sb[:].ap[0], [P, kq], [1, P]])
                nc.vector.tensor_tensor(out=oh3d, in0=iota3d, in1=dstl3d,
                                        op=mybir.AluOpType.is_equal)

                ncols = len(gts) * P
                ps1 = ppool.tile([P, 512], F32, tag="ps1", name=f"ps1_{gi}",
                                 space="PSUM", bufs=3)
                for ti, t in enumerate(gts):
                    chunks = ([chunk_off[(t, 0)] + i for i in range(KL[t])]
                              + [chunk_off[(t, 1)] + i for i in range(KH[t])])
                    for ci, ch in enumerate(chunks):
                        nc.tensor.matmul(
                            out=ps1[:, ti * P:(ti + 1) * P],
                            lhsT=g_sb[:, ch * P:(ch + 1) * P],
                            rhs=oh_sb[:, ch * P:(ch + 1) * P],
                            start=(ci == 0), stop=(ci == len(chunks) - 1))

                # fused divide + eviction (agg^T in fp16)
                col0 = gts[0] * P
                aggT_sb = wpool.tile([P, ncols], F16, tag="aggT",
                                     name=f"agT{gi}")
                nc.vector.tensor_tensor(
                    out=aggT_sb[:], in0=ps1[:, :ncols],
                    in1=rcrep_sb[:, col0:col0 + ncols],
                    op=mybir.AluOpType.mult)

                ps2 = ppool.tile([P, 512], F32, tag="ps2", name=f"ps2_{gi}",
                                 space="PSUM")
                nc.tensor.matmul(out=ps2[:, :ncols], lhsT=wn_sb[:],
                                 rhs=aggT_sb[:], start=True, stop=False)
                nc.tensor.matmul(out=ps2[:, :ncols], lhsT=ws_sb[:],
                                 rhs=xdstT_sb[:, col0:col0 + ncols],
                                 start=False, stop=True)
                # bias fused into ACT eviction
                o_sb = wpool.tile([P, ncols], F32, tag="osb", name=f"o{gi}")
                nc.scalar.activation(
                    out=o_sb[:], in_=ps2[:, :ncols],
                    func=mybir.ActivationFunctionType.Identity,
                    bias=bsumT_sb[:])
                cols = min(ncols, DST_PER_CORE - col0)
                nc.scalar.dma_start(out=out_d[:, col0:col0 + cols],
                                    in_=o_sb[:, :cols])

    nc.finalize()

    in_maps = [{
        "xlo": x_lo, "xhi": x_hi, "idx": idx_all[c], "dstl": dstl_all[c],
        "xdstT": xdstT[c], "iota": iota, "rcr": rcnt_row[c], "wn": wn,
        "ws": ws, "bsumT": bsumT,
    } for c in range(N_CORES)]

    trace = os.environ.get("BSAGE_TRACE", "0") == "1"
    res = run_bass_kernel_spmd(nc, in_maps, core_ids=list(range(N_CORES)),
                               trace=trace)
    out = np.empty((N_DST, OUT), np.float32)
    for c in range(N_CORES):
        out[c * DST_PER_CORE:(c + 1) * DST_PER_CORE] = \
            res.results[c]["out"][:, :DST_PER_CORE].T
    if trace:
        build_and_run.last_exec_ns = res.exec_time_ns
    return out


def kernel(x_src, x_dst, edge_src, edge_dst, num_dst, W_neigh, b_neigh,
           W_self, b_self):
    x_src = np.asarray(x_src, dtype=np.float32)
    x_dst = np.asarray(x_dst, dtype=np.float32)
    edge_src = np.asarray(edge_src).astype(np.int64)
    edge_dst = np.asarray(edge_dst).astype(np.int64)
    W_neigh = np.asarray(W_neigh, dtype=np.float32)
    b_neigh = np.asarray(b_neigh, dtype=np.float32)
    W_self = np.asarray(W_self, dtype=np.float32)
    b_self = np.asarray(b_self, dtype=np.float32)
    return build_and_run(x_src, x_dst, edge_src, edge_dst, W_neigh, b_neigh,
                         W_self, b_self)


# revision 3
# speedup vs baseline: 1.2576x; 1.2576x over previous
"""BipartiteSAGEConv Trainium2 kernel (v3).

Strategy: destination-sharded, zero collectives, fully transposed dataflow.
- Host: partition edges by destination across 8 cores (6250 dsts each),
  group per 128-dst tile, split by src half (int16 index limit), pad to
  uniform chunk structure across cores (SPMD: one program, 8 data sets).
  Host also computes 1/deg per dst (index-only preprocessing).
- Device per core: dma_gather pulls per-edge src rows HBM->SBUF (fp16).
  Gathers cover contiguous chunk spans per (tile-group, half) and are
  split into small (512-row) instructions so 2-3 fit in each SWDGE
  queue's descriptor ring -> per-queue pipelining of descriptor-gen with
  DMA drain. Scatter-add via one-hot matmul with the GATHERED ROWS AS
  STATIONARY so the accumulator is agg^T directly (no PE transposes);
  divide fused into the PSUM eviction (DVE multiply by a replicated
  1/deg row); two linear layers as W^T @ x matmuls; bias fused into the
  ACT-engine eviction; output stored transposed, un-transposed on host.
"""

import os
import sys
import types

import numpy as np

N_SRC = 50000
N_DST = 50000
E = 800000
D = 128
OUT = 128
N_CORES = 8
P = 128
DST_PER_CORE = N_DST // N_CORES          # 6250
TILES = (DST_PER_CORE + P - 1) // P      # 49
HALF = 25000                             # int16 index limit split
MAX_ROWS = int(os.environ.get("BSAGE_ROWS", "512"))  # rows per gather inst
GROUP = int(os.environ.get("BSAGE_GROUP", "4"))      # tiles per psum group
SKIP_PAD = os.environ.get("BSAGE_SKIP", "1") == "1"


def _install_ntff_hook():
    try:
        import antenv
        if "antenv.axon_hooks" in sys.modules:
            return
        mod = types.ModuleType("antenv.axon_hooks")
        _h = [None]
        mod.set_axon_ntff_profile_hook = lambda h: _h.__setitem__(0, h)
        mod.get_axon_ntff_profile_hook = lambda: _h[0]
        sys.modules["antenv.axon_hooks"] = mod
        antenv.axon_hooks = mod
        from trn_agent_boot.trn_boot import _ntff_profile_via_ctypes
        mod.set_axon_ntff_profile_hook(
            _ntff_profile_via_ctypes("/opt/axon/libaxon_pjrt.so"))
    except Exception:
        pass


def _prep_core(edge_src, edge_dst, core):
    """Per-core: for each (tile, half) the (src, dstl) edge lists."""
    lo = core * DST_PER_CORE
    m = (edge_dst >= lo) & (edge_dst < lo + DST_PER_CORE)
    es = edge_src[m]
    ed = edge_dst[m] - lo
    order = np.argsort(ed, kind="stable")
    es, ed = es[order], ed[order]
    tiles = []
    tile_id = ed >> 7
    bounds = np.searchsorted(tile_id, np.arange(TILES + 1))
    for t in range(TILES):
        a, b = bounds[t], bounds[t + 1]
        s, dl = es[a:b], ed[a:b] - t * P
        is_lo = s < HALF
        tiles.append((s[is_lo], dl[is_lo], s[~is_lo] - HALF, dl[~is_lo]))
    return tiles


def _wrap_idx(idx_flat):
    """dma_gather wrapped index layout: index j at partition j%16, col j//16,
    replicated across the 8 gpsimd cores (partition groups of 16)."""
    n = len(idx_flat)
    w = idx_flat.reshape(n // 16, 16).T          # [16, n/16]
    return np.tile(w, (8, 1))                    # [128, n/16]


def build_and_run(x_src, x_dst, edge_src, edge_dst, W_neigh, b_neigh,
                  W_self, b_self):
    _install_ntff_hook()
    from concourse import bacc, bass, mybir, tile
    from concourse.bass_utils import run_bass_kernel_spmd

    F32 = mybir.dt.float32
    F16 = mybir.dt.float16

    # ---------- host-side sharding / layout ----------
    per_core_tiles = [_prep_core(edge_src, edge_dst, c) for c in range(N_CORES)]

    # uniform chunk counts across cores (SPMD: one program, 8 data sets)
    KL = [max(max(1, -(-len(per_core_tiles[c][t][0]) // P))
              for c in range(N_CORES)) for t in range(TILES)]
    KH = [max(max(1, -(-len(per_core_tiles[c][t][2]) // P))
              for c in range(N_CORES)) for t in range(TILES)]

    def _r16(n):
        return max(16, -(-n // 16) * 16)

    # tile groups sharing one g buffer / psum accumulator
    GROUPS = [list(range(g, min(g + GROUP, TILES)))
              for g in range(0, TILES, GROUP)]
    NG = len(GROUPS)
    # chunk layout within a group buffer: per half, tiles concatenated:
    # [t0_lo|t1_lo|...|t0_hi|t1_hi|...]
    chunk_off = {}
    group_chunks = []
    span_of = {}                      # (gi, half) -> (chunk_base, n_chunks)
    for gi, gts in enumerate(GROUPS):
        off = 0
        for half in (0, 1):
            base = off
            for t in gts:
                chunk_off[(t, half)] = off
                off += (KL if half == 0 else KH)[t]
            span_of[(gi, half)] = (base, off - base)
        group_chunks.append(off)
    GCMAX = max(group_chunks)
    NCH = sum(KL) + sum(KH)
    # dstl column base per (tile, half), following the group layout order
    cbase_t = {}
    gbase = [0]
    _c = 0
    for gi, gts in enumerate(GROUPS):
        for half in (0, 1):
            for t in gts:
                cbase_t[(t, half)] = _c
                _c += (KL if half == 0 else KH)[t]
        gbase.append(_c)

    # valid (gathered) rows per (gi, half) span: full span, minus trailing
    # pad of the span's LAST tile rounded to 16 (uniform across cores)
    span_valid = {}
    for gi, gts in enumerate(GROUPS):
        for half in (0, 1):
            base, nch = span_of[(gi, half)]
            t_last = gts[-1]
            K = (KL if half == 0 else KH)[t_last]
            r = _r16(max(len(per_core_tiles[c][t_last][2 * half])
                         for c in range(N_CORES)))
            v = nch * P - (K * P - r) if SKIP_PAD else nch * P
            span_valid[(gi, half)] = v

    # gather plan: split each (group, half) span into <= MAX_ROWS rows.
    # entries: (gi, half, chunk_off_in_group, n_chunks, idx_col_base, valid)
    gathers = []
    idx_cols = 0
    for gi in range(NG):
        for half in (0, 1):
            base, nch = span_of[(gi, half)]
            vspan = span_valid[(gi, half)]
            k_done = 0
            while k_done < nch:
                k = min(nch - k_done, MAX_ROWS // P)
                valid = min(max(0, vspan - k_done * P), k * P)
                gathers.append((gi, half, base + k_done, k, idx_cols, valid))
                idx_cols += k * 8
                k_done += k
    IDXCOLS = idx_cols

    # per-core data arrays
    idx_all = np.zeros((N_CORES, P, IDXCOLS), np.int16)
    dstl_all = np.zeros((N_CORES, P, NCH), np.float32)
    for c in range(N_CORES):
        # per-(group,half) flat src streams in chunk-layout order
        for gi, gts in enumerate(GROUPS):
            for half in (0, 1):
                base, nch = span_of[(gi, half)]
                vspan = span_valid[(gi, half)]
                parts = []
                for t in gts:
                    tl = per_core_tiles[c][t]
                    s = tl[0] if half == 0 else tl[2]
                    dl = tl[1] if half == 0 else tl[3]
                    K = (KL if half == 0 else KH)[t]
                    n = K * P
                    sp = np.zeros(n, np.int16)
                    sp[:len(s)] = s.astype(np.int16)
                    dp = np.full(n, -1.0, np.float32)
                    dp[:len(dl)] = dl.astype(np.float32)
                    cb = cbase_t[(t, half)]
                    dstl_all[c][:, cb:cb + K] = dp.reshape(K, P).T
                    parts.append(sp)
                stream = np.concatenate(parts)
                stream[vspan:] = -1     # trailing skip (uniform)
                for (gi2, h2, off, k, colb, valid) in gathers:
                    if gi2 != gi or h2 != half:
                        continue
                    k0 = off - base
                    rows = stream[k0 * P:(k0 + k) * P]
                    idx_all[c][:, colb:colb + k * 8] = _wrap_idx(rows)

    x_lo = np.ascontiguousarray(x_src[:HALF]).astype(np.float16)
    x_hi = np.ascontiguousarray(x_src[HALF:]).astype(np.float16)
    xdstT = np.zeros((N_CORES, P, TILES * P), np.float16)
    for c in range(N_CORES):
        shard = x_dst[c * DST_PER_CORE:(c + 1) * DST_PER_CORE]
        xdstT[c][:, :DST_PER_CORE] = shard.T.astype(np.float16)
    cnt = np.bincount(edge_dst.astype(np.int64), minlength=N_DST).astype(
        np.float32)
    rcnt = 1.0 / np.maximum(cnt, 1.0)
    rcnt_row = np.ones((N_CORES, 1, TILES * P), np.float32)
    for c in range(N_CORES):
        rcnt_row[c][0, :DST_PER_CORE] = rcnt[c * DST_PER_CORE:
                                             (c + 1) * DST_PER_CORE]
    iota = np.tile(np.arange(P, dtype=np.float32), (P, 1))
    wn = W_neigh.astype(np.float16)
    ws = W_self.astype(np.float16)
    bsumT = (b_neigh + b_self).astype(np.float32)[:, None]  # [128,1]

    # idx column split: groups 0-1 in the first (early) tile
    cols_e = max((g[4] + g[3] * 8) for g in gathers if g[0] <= 1)

    # ---------- device program ----------
    nc = bacc.Bacc("TRN2", target_bir_lowering=False, debug=False,
                   num_devices=N_CORES, num_swdge_queues=4)
    xlo_d = nc.dram_tensor("xlo", [HALF, D], F16, kind="ExternalInput").ap()
    xhi_d = nc.dram_tensor("xhi", [HALF, D], F16, kind="ExternalInput").ap()
    idx_d = nc.dram_tensor("idx", [P, IDXCOLS], mybir.dt.int16,
                           kind="ExternalInput").ap()
    dstl_d = nc.dram_tensor("dstl", [P, NCH], F32, kind="ExternalInput").ap()
    xdstT_d = nc.dram_tensor("xdstT", [P, TILES * P], F16,
                             kind="ExternalInput").ap()
    iota_d = nc.dram_tensor("iota", [P, P], F32, kind="ExternalInput").ap()
    rcr_d = nc.dram_tensor("rcr", [1, TILES * P], F32,
                           kind="ExternalInput").ap()
    wn_d = nc.dram_tensor("wn", [D, OUT], F16, kind="ExternalInput").ap()
    ws_d = nc.dram_tensor("ws", [D, OUT], F16, kind="ExternalInput").ap()
    bsumT_d = nc.dram_tensor("bsumT", [OUT, 1], F32, kind="ExternalInput").ap()
    out_d = nc.dram_tensor("out", [P, TILES * P], F32,
                           kind="ExternalOutput").ap()

    RTILES = -(-TILES * P // 512)

    with tile.TileContext(nc) as tc:
        with (
            tc.tile_pool(name="const", bufs=1) as cpool,
            tc.tile_pool(name="work", bufs=4) as wpool,
            tc.tile_pool(name="psum", bufs=2, space="PSUM") as ppool,
        ):
            idxe_sb = cpool.tile([P, cols_e], mybir.dt.int16)
            idxr_sb = cpool.tile([P, IDXCOLS - cols_e], mybir.dt.int16)
            dstl_sb = cpool.tile([P, NCH], F32)
            xdstT_sb = cpool.tile([P, TILES * P], F16)
            iota_sb = cpool.tile([P, P], F32)
            rcr_sb = cpool.tile([1, TILES * P], F32)
            rcrep_sb = cpool.tile([P, TILES * P], F32)
            wn_sb = cpool.tile([D, OUT], F16)
            ws_sb = cpool.tile([D, OUT], F16)
            bsumT_sb = cpool.tile([OUT, 1], F32)
            ones_sb = cpool.tile([1, P], F32)

            def idx_ref(colb, ncols):
                if colb < cols_e:
                    return idxe_sb[:, colb:colb + ncols]
                return idxr_sb[:, colb - cols_e:colb - cols_e + ncols]

            # critical path first: early idx, then everything else
            nc.sync.dma_start(out=idxe_sb[:], in_=idx_d[:, :cols_e])
            nc.sync.dma_start(out=idxr_sb[:], in_=idx_d[:, cols_e:])
            nc.sync.dma_start(out=dstl_sb[:], in_=dstl_d[:])
            nc.sync.dma_start(out=iota_sb[:], in_=iota_d[:])
            nc.scalar.dma_start(out=rcr_sb[:], in_=rcr_d[:])
            nc.scalar.dma_start(out=wn_sb[:], in_=wn_d[:])
            nc.scalar.dma_start(out=ws_sb[:], in_=ws_d[:])
            nc.scalar.dma_start(out=bsumT_sb[:], in_=bsumT_d[:])
            nc.scalar.dma_start(out=xdstT_sb[:], in_=xdstT_d[:])
            nc.vector.memset(ones_sb[:], 1.0)

            # replicate rcnt row across partitions via PE (ones^T @ rcr)
            for j in range(RTILES):
                a = j * 512
                b = min((j + 1) * 512, TILES * P)
                psr = ppool.tile([P, 512], F32, tag="psr", name=f"psr{j}",
                                 space="PSUM")
                nc.tensor.matmul(out=psr[:, :b - a], lhsT=ones_sb[:],
                                 rhs=rcr_sb[:, a:b], start=True, stop=True)
                nc.scalar.copy(out=rcrep_sb[:, a:b], in_=psr[:, :b - a])

            g_by_group = [[] for _ in range(NG)]
            for g in gathers:
                g_by_group[g[0]].append(g)

            gq = [0]
            for gi, gts in enumerate(GROUPS):
                gcols = group_chunks[gi] * P
                g_sb = wpool.tile([P, GCMAX * P], F16, tag="g", name=f"g{gi}",
                                  bufs=3)
                # memset once per physical buffer to avoid NaN in skipped
                # trailing rows (one-hot is 0 there; 0*NaN would poison psum)
                if gi < 3:
                    nc.vector.memset(g_sb[:], 0.0)
                for (_, half, off, k, colb, valid) in g_by_group[gi]:
                    t_ap = g_sb[:]
                    out3d = bass.AP(t_ap.tensor, t_ap.offset + off * P,
                                    [t_ap.ap[0], [P, k], [1, P]])
                    nc.gpsimd.dma_gather(
                        out3d,
                        (xlo_d if half == 0 else xhi_d)[:],
                        idx_ref(colb, k * 8),
                        k * P,
                        valid,
                        D,
                        queue_num=(gq[0] % 4),
                    )
                    gq[0] += 1

                # one-hot for the whole group in one DVE instruction
                kq = group_chunks[gi]
                oh_sb = wpool.tile([P, GCMAX * P], F16, tag="oh",
                                   name=f"oh{gi}", bufs=3)
                i_ap = iota_sb[:]
                iota3d = bass.AP(i_ap.tensor, i_ap.offset,
                                 [i_ap.ap[0], [0, kq], [i_ap.ap[1][0], P]])
                d_ap = dstl_sb[:]
                dstl3d = bass.AP(d_ap.tensor, d_ap.offset + int(gbase[gi]),
                                 [d_ap.ap[0], [d_ap.ap[1][0], kq], [0, P]])
                oh3d = bass.AP(oh_sb[:].tensor, oh_sb[:].offset,
                               [oh_sb[:].ap[0], [P, kq], [1, P]])
                nc.vector.tensor_tensor(out=oh3d, in0=iota3d, in1=dstl3d,
                                        op=mybir.AluOpType.is_equal)

                ncols = len(gts) * P
                ps1 = ppool.tile([P, 512], F32, tag="ps1", name=f"ps1_{gi}",
                                 space="PSUM", bufs=3)
                for ti, t in enumerate(gts):
                    chunks = ([chunk_off[(t, 0)] + i for i in range(KL[t])]
                              + [chunk_off[(t, 1)] + i for i in range(KH[t])])
                    for ci, ch in enumerate(chunks):
                        nc.tensor.matmul(
                            out=ps1[:, ti * P:(ti + 1) * P],
                            lhsT=g_sb[:, ch * P:(ch + 1) * P],
                            rhs=oh_sb[:, ch * P:(ch + 1) * P],
                            start=(ci == 0), stop=(ci == len(chunks) - 1))

                # fused divide + eviction (agg^T in fp16)
                col0 = gts[0] * P
                aggT_sb = wpool.tile([P, ncols], F16, tag="aggT",
                                     name=f"agT{gi}")
                nc.vector.tensor_tensor(
                    out=aggT_sb[:], in0=ps1[:, :ncols],
                    in1=rcrep_sb[:, col0:col0 + ncols],
                    op=mybir.AluOpType.mult)

                ps2 = ppool.tile([P, 512], F32, tag="ps2", name=f"ps2_{gi}",
                                 space="PSUM")
                nc.tensor.matmul(out=ps2[:, :ncols], lhsT=wn_sb[:],
                                 rhs=aggT_sb[:], start=True, stop=False)
                nc.tensor.matmul(out=ps2[:, :ncols], lhsT=ws_sb[:],
                                 rhs=xdstT_sb[:, col0:col0 + ncols],
                                 start=False, stop=True)
                # bias fused into ACT eviction
                o_sb = wpool.tile([P, ncols], F32, tag="osb", name=f"o{gi}")
                nc.scalar.activation(
                    out=o_sb[:], in_=ps2[:, :ncols],
                    func=mybir.ActivationFunctionType.Identity,
                    bias=bsumT_sb[:])
                cols = min(ncols, DST_PER_CORE - col0)
                nc.scalar.dma_start(out=out_d[:, col0:col0 + cols],
                                    in_=o_sb[:, :cols])

    nc.finalize()

    in_maps = [{
        "xlo": x_lo, "xhi": x_hi, "idx": idx_all[c], "dstl": dstl_all[c],
        "xdstT": xdstT[c], "iota": iota, "rcr": rcnt_row[c], "wn": wn,
        "ws": ws, "bsumT": bsumT,
    } for c in range(N_CORES)]

    trace = os.environ.get("BSAGE_TRACE", "0") == "1"
    res = run_bass_kernel_spmd(nc, in_maps, core_ids=list(range(N_CORES)),
                               trace=trace)
    out = np.empty((N_DST, OUT), np.float32)
    for c in range(N_CORES):
        out[c * DST_PER_CORE:(c + 1) * DST_PER_CORE] = \
            res.results[c]["out"][:, :DST_PER_CORE].T
    if trace:
        build_and_run.last_exec_ns = res.exec_time_ns
    return out


def kernel(x_src, x_dst, edge_src, edge_dst, num_dst, W_neigh, b_neigh,
           W_self, b_self):
    x_src = np.asarray(x_src, dtype=np.float32)
    x_dst = np.asarray(x_dst, dtype=np.float32)
    edge_src = np.asarray(edge_src).astype(np.int64)
    edge_dst = np.asarray(edge_dst).astype(np.int64)
    W_neigh = np.asarray(W_neigh, dtype=np.float32)
    b_neigh = np.asarray(b_neigh, dtype=np.float32)
    W_self = np.asarray(W_self, dtype=np.float32)
    b_self = np.asarray(b_self, dtype=np.float32)
    return build_and_run(x_src, x_dst, edge_src, edge_dst, W_neigh, b_neigh,
                         W_self, b_self)
